# revision 4
# baseline (speedup 1.0000x reference)
"""Trainium2 Bass kernel for nn_GatedBlock (moe_routing).

Math (reference collapses): the (NB,BS,BS) reshape of weight maps block k to
rows [128k, 128k+128) of weight, so
    out[b, i] = g[b, i // 128] * (x @ W.T)[b, i] + bias[i]
with g = sigmoid(x @ gate_w + gate_b), bottom-8 of 16 gates zeroed per row.

Sharding: output-dim (i) split 8 ways -> 256 rows of W (= 2 gate blocks) per
core.  Per-core inputs (all k-tile-major, partition-contiguous rows):
  small (128, KT, 96) bf16  [x_hi | x_lo | gw_hi | gw_lo]  (3KB rows)
  rhs   (128, KT, 256) bf16 W_shard.T                      (512B/k-tile rows)
  epi   (32, 272) f32       [bias_shard bcast | gate_b[perm] bcast]

Everything runs in bf16 on the tensor engine:
  gate:  16 mms, lhsT = [x_hi|x_lo] (M=64), rhs = [gw_hi|gw_lo] (N=32);
         logits = ps[0:32,0:16] + ps[0:32,16:32] + ps[32:64,0:16] recovers
         x@gw to ~1e-5 (bf16 products are exact, fp32 PSUM accumulate, only
         the x_lo*gw_lo term is dropped) — top-8 ranking is safe (min gate
         margin 3.4e-4).
  main:  16 mms, lhsT = x_hi (M=32), rhs = W k-tile (N=256) — one wide mm
         per k-tile amortizes the ~50ns per-instruction overhead.
W in bf16 gives 1.9e-3 rel err vs the 2e-2 gate and halves the dominant DMA.
DMA: one small DMA + one W-segment DMA per HWDGE queue (big partition rows
amortize the ~60ns/packet engine pacing and the ~0.6us per-DMA turnaround).
"""

import sys

for _p in ("/opt/trn_rl_repo", "/root/.axon_site/_ro/trn_rl_repo"):
    if _p not in sys.path:
        sys.path.append(_p)

import os as _os

import numpy as np

B = 32          # batch
D = 2048        # model dim
NB = 16         # gate blocks
BLK = D // NB   # 128 output rows per gate block
N_CORES = 8
NOUT = D // N_CORES       # 256 output cols per core
KT = D // 128             # 16 k-tiles
NSM = 96                  # small cols: 64 x (hi|lo) + 32 gw (hi|lo)

# k split points for the W DMA segments (round-robin over queues, starting
# with the queue that does NOT carry the small array)
SPLITS = [int(v) for v in _os.environ.get("GATED_SPLITS", "8").split(",") if v]
DMA_ENGS = _os.environ.get("GATED_ENGS", "sync,scalar").split(",")

_compiled = {}


def _build():
    import concourse.bacc as bacc
    import concourse.tile as tile
    import concourse.mybir as mybir

    f32 = mybir.dt.float32
    bf16 = mybir.dt.bfloat16

    nc = bacc.Bacc("TRN2", target_bir_lowering=False, debug=False,
                   num_devices=N_CORES)

    small_d = nc.dram_tensor("small", [128, KT, NSM], bf16, kind="ExternalInput")
    rhs_d = nc.dram_tensor("rhs", [128, KT, NOUT], bf16, kind="ExternalInput")
    epi_d = nc.dram_tensor("epi", [B, NOUT + NB], f32, kind="ExternalInput")
    out_d = nc.dram_tensor("out", [B, NOUT], f32, kind="ExternalOutput")

    with tile.TileContext(nc) as tc:
        with (
            tc.tile_pool(name="sb", bufs=1) as sb,
            tc.tile_pool(name="ps", bufs=1, space="PSUM") as psp,
        ):
            small = sb.tile([128, KT, NSM], bf16, name="small_sb", tag="small_sb")
            rhs = sb.tile([128, KT, NOUT], bf16, name="rhs_sb", tag="rhs_sb")
            epi = sb.tile([B, NOUT + NB], f32, name="epi_sb", tag="epi_sb")
            t1 = sb.tile([B, NB], f32, name="t1", tag="t1")
            graw = sb.tile([B, NB], f32, name="graw", tag="graw")
            g = sb.tile([B, NB], f32, name="g", tag="g")
            m8 = sb.tile([B, 8], f32, name="m8", tag="m8")
            rep = sb.tile([B, NB], f32, name="rep", tag="rep")
            gk = sb.tile([B, NB], f32, name="gk", tag="gk")
            outt = sb.tile([B, NOUT], f32, name="outt", tag="outt")
            ps_g = psp.tile([2 * B, 2 * NB], f32, name="ps_g", tag="ps_g")
            ps_m = psp.tile([B, NOUT], f32, name="ps_m", tag="ps_m")

            engs = [getattr(nc, e) for e in DMA_ENGS]

            # queue 0: the small array (gate + stationary data) first;
            # queue 1: epi then the first W segment.  W segments round-robin
            # starting on queue 1 so queue 0's small lands first.
            engs[0].dma_start(small[:], small_d.ap())
            engs[1 % len(engs)].dma_start(epi[:], epi_d.ap())
            bounds = [0] + SPLITS + [KT]
            for si in range(len(bounds) - 1):
                k0, k1 = bounds[si], bounds[si + 1]
                engs[(si + 1) % len(engs)].dma_start(
                    rhs[:, k0:k1, :], rhs_d.ap()[:, k0:k1, :])

            # gate linear: M=64 ([x_hi|x_lo]) x N=32 ([gw_hi|gw_lo])
            for t in range(KT):
                nc.tensor.matmul(
                    ps_g[:], small[:, t, 0:64], small[:, t, 64:NSM],
                    start=(t == 0), stop=(t == KT - 1),
                )

            # logits = hi*hi + hi*lo + lo*hi + gate_b, then sigmoid/top-8
            # (chained so each tensor_tensor reads at most one PSUM input)
            nc.vector.tensor_add(t1[:], ps_g[0:B, 0:NB], epi[:, NOUT:NOUT + NB])
            nc.vector.tensor_add(t1[:], ps_g[0:B, NB:2 * NB], t1[:])
            nc.vector.tensor_add(graw[:], ps_g[B:2 * B, 0:NB], t1[:])
            nc.scalar.activation(g[:], graw[:],
                                 mybir.ActivationFunctionType.Sigmoid)
            nc.vector.max(m8[:], g[:])
            nc.vector.match_replace(rep[:], m8[:], g[:], 0.0)
            nc.vector.tensor_sub(gk[:], g[:], rep[:])

            # main matmul: one N=256 mm per k-tile, x_hi stationary
            for t in range(KT):
                nc.tensor.matmul(
                    ps_m[:], small[:, t, 0:B], rhs[:, t, :],
                    start=(t == 0), stop=(t == KT - 1),
                )

            # out = psum * g[block] + bias, in halves so the first store
            # overlaps the second STT
            nh = NOUT // BLK
            for h in range(nh):
                sl = slice(h * BLK, (h + 1) * BLK)
                nc.vector.scalar_tensor_tensor(
                    outt[:, sl], ps_m[:, sl], gk[:, h:h + 1], epi[:, sl],
                    mybir.AluOpType.mult, mybir.AluOpType.add,
                )
                engs[(h + 1) % len(engs)].dma_start(out_d.ap()[:, sl], outt[:, sl])

    nc.compile()
    return nc


def _tile_major(a):
    """(D, n) -> (128, KT, n) k-tile-major contiguous."""
    n = a.shape[1]
    return np.ascontiguousarray(a.reshape(KT, 128, n).transpose(1, 0, 2))


def _hi_lo(a):
    import ml_dtypes
    hi = a.astype(ml_dtypes.bfloat16)
    lo = (a - hi.astype(np.float32)).astype(ml_dtypes.bfloat16)
    return hi, lo


def build_in_maps(x, gate_w, gate_b, weight, bias):
    import ml_dtypes

    x = np.asarray(x, dtype=np.float32)
    gate_w = np.asarray(gate_w, dtype=np.float32)
    gate_b = np.asarray(gate_b, dtype=np.float32)
    weight = np.asarray(weight, dtype=np.float32)
    bias = np.asarray(bias, dtype=np.float32)

    x_hi, x_lo = _hi_lo(np.ascontiguousarray(x.T))               # (2048, 32)
    in_maps = []
    for c in range(N_CORES):
        perm = [2 * c, 2 * c + 1] + [k for k in range(NB)
                                     if k not in (2 * c, 2 * c + 1)]
        gw_hi, gw_lo = _hi_lo(gate_w[:, perm])                   # (2048, 16)
        small = np.concatenate([x_hi, x_lo, gw_hi, gw_lo], axis=1)  # (2048, 96)
        w_shard = np.ascontiguousarray(weight[c * NOUT:(c + 1) * NOUT, :].T)
        epi = np.concatenate([
            np.broadcast_to(bias[c * NOUT:(c + 1) * NOUT], (B, NOUT)),
            np.broadcast_to(gate_b[perm], (B, NB)),
        ], axis=1).astype(np.float32)
        in_maps.append({
            "small": _tile_major(small),
            "rhs": _tile_major(w_shard.astype(ml_dtypes.bfloat16)),
            "epi": np.ascontiguousarray(epi),
        })
    return in_maps


def _ensure_ntff_hook():
    """If a caller sets BASS_TRACE, run_bass_kernel_spmd imports
    antenv.axon_hooks, which is missing in this image; provide a working
    ctypes-backed stub so tracing degrades gracefully instead of raising."""
    try:
        from antenv.axon_hooks import get_axon_ntff_profile_hook  # noqa: F401
        return
    except ImportError:
        pass
    import contextlib
    import ctypes
    import types

    try:
        lib = ctypes.CDLL("/opt/axon/libaxon_pjrt.so")
        assert hasattr(lib, "axon_start_nrt_profile")
        lib.axon_start_nrt_profile.argtypes = [
            ctypes.POINTER(ctypes.c_int64), ctypes.c_size_t]
        lib.axon_start_nrt_profile.restype = ctypes.c_int64
        lib.axon_stop_nrt_profile.argtypes = [ctypes.c_char_p]
        lib.axon_stop_nrt_profile.restype = ctypes.c_int64

        @contextlib.contextmanager
        def _hook(output_dir, device_ids):
            import jax
            jax.devices()
            if device_ids:
                ids = (ctypes.c_int64 * len(device_ids))(*device_ids)
                rc = lib.axon_start_nrt_profile(ids, len(device_ids))
            else:
                rc = lib.axon_start_nrt_profile(None, 0)
            if rc != 0:
                raise RuntimeError(f"axon_start_nrt_profile rc={rc}")
            try:
                yield
            finally:
                lib.axon_stop_nrt_profile(str(output_dir).encode())

        hook = _hook
    except Exception:
        hook = None

    mod = types.ModuleType("antenv.axon_hooks")
    mod.get_axon_ntff_profile_hook = lambda: hook
    mod.set_axon_ntff_profile_hook = lambda h: None
    sys.modules["antenv.axon_hooks"] = mod


MODE = "v3"  # single variant; kept for test.py compatibility


def kernel(x, gate_w, gate_b, weight, bias):
    _ensure_ntff_hook()
    from concourse.bass_utils import run_bass_kernel_spmd

    if MODE not in _compiled:
        _compiled[MODE] = _build()
    nc = _compiled[MODE]

    in_maps = build_in_maps(x, gate_w, gate_b, weight, bias)
    res = run_bass_kernel_spmd(nc, in_maps, list(range(N_CORES)))
    out = np.concatenate([res.results[c]["out"] for c in range(N_CORES)], axis=1)
    return out.astype(np.float32)
